# revision 13
# baseline (speedup 1.0000x reference)
# Trainium2 Bass kernel for nn_FHNTritonAttention: causal attention with an
# FHN (FitzHugh-Nagumo) gate on the attention probabilities.
#
# Math note that shapes the whole kernel: attn_energy = softmax(scores).sum(-1)
# is ~1.0 for every row (softmax rows sum to 1), so stimulus_normed == 1,
# threshold_gate == sigmoid(5), and the FHN recurrence collapses to one
# constant per run. The gate multiplies each probability row by a constant c
# and the subsequent renormalization divides it back out:
#   p'' = p*c / (c*S + 1e-8) = p / (S + 1e-8/c),  S = row sum ~= 1.
# So the entire FHN block reduces to scaling the output by
# f0 = 1/(1 + 1e-8/c0), computed on host from (a, b, dt) and folded into Wout.
#
# Device kernel (SPMD over 8 cores; core = (batch, 4-head group)), emitted as
# one software-pipelined stream so the in-order PE queue never drains:
#   A(n): qkT = Wqk_slice @ x.T (feature on partitions) and v = x @ Wv_slice.T
#         for t-chunk n (512 cols), contraction over D in 8 psum passes.
#         v lands in v_pad tiles [128, h*128] whose upper 64 columns per head
#         are constant 1.0 (memset at start): the PV matmul then produces the
#         softmax denominator Z replicated on psum partitions 64..127.
#   B(g): attention for q-tile g. Per head pair, s-chunks run through a
#         depth-2 software pipeline (score matmul for chunk j+1 issues before
#         the PV matmul of chunk j) so the exp latency on the Scalar engine is
#         hidden. Score matmuls skip the fully-masked leading w0 columns of
#         diagonal chunks. Normalization: one DVE reciprocal of psum rows
#         64..127 ([64, 512]) + one fused multiply that evicts+normalizes the
#         psum into the attn tile — no partition broadcast, no zrow copies.
#   Y(g): out-projection of q-tile g -> DMA out. Emission order
#         A0 B0 A1 Y0 B1 A2 Y1 B2 A3 Y2 B3 Y3 keeps every engine streaming.
#   Host sums the 4 partial products per batch and transposes back.
#
# Matmuls run in bf16 (inputs pre-cast on host; fp32 PSUM accumulation).

import math
import os

import numpy as np

N_HEADS = 16
HEAD_DIM = 64
THRESHOLD = 0.5
TAU = 12.5
N_FHN_STEPS = 2

N_CORES = 8
HEADS_PER_CORE = 4  # cores 0-3 -> batch 0, cores 4-7 -> batch 1

ATTN_DTYPE = os.environ.get("KERNEL_ATTN_DTYPE", "bfloat16")

LAST_RUN = {}  # filled with exec_time_ns etc. when KERNEL_TRACE is set

_PROGRAM_CACHE = {}


def _fhn_scale(a, b, dt):
    """Host-side replica of the reference's gate math at attn_energy == 1."""
    a = float(a)
    b = float(b)
    dt = float(dt)
    sig5 = 1.0 / (1.0 + math.exp(-(1.0 - THRESHOLD) * 10.0))
    i0 = 1.0 * (0.1 + 0.9 * sig5)
    v = 0.0
    w = 0.0
    for _ in range(N_FHN_STEPS):
        v = v + dt * (v - v**3 / 3.0 - w + i0)
        w = (w + (dt / TAU) * (v + a)) / (1.0 + (dt / TAU) * b)
    gate = 1.0 / (1.0 + math.exp(-v))
    c0 = 0.5 + 0.5 * gate
    return c0 / (c0 + 1e-8)


def _build_program(T, D, H_per_core, hd):
    import concourse.mybir as mybir
    import concourse.tile as tile
    from concourse import bacc

    f32 = mybir.dt.float32
    at_dt = getattr(mybir.dt, ATTN_DTYPE)
    P = 128
    QT = 512   # q tile width (free dim of score/PV matmuls)
    SC = 128   # s chunk depth (contraction of PV, partitions of scoresT)
    K_D = D // P
    QK_ROWS = 2 * H_per_core * hd
    V_COLS = H_per_core * hd
    C = H_per_core * hd
    n_qt = T // QT
    n_pairs = H_per_core // 2

    nc = bacc.Bacc("TRN2", target_bir_lowering=False, debug=False,
                   num_devices=N_CORES)

    xt_d = nc.declare_dram_parameter("xt", [D, T], at_dt, isOutput=False)
    wqkt_d = nc.declare_dram_parameter("wqkt", [D, QK_ROWS], at_dt, isOutput=False)
    wvt_d = nc.declare_dram_parameter("wvt", [D, V_COLS], at_dt, isOutput=False)
    wot_d = nc.declare_dram_parameter("wot", [C, D], at_dt, isOutput=False)
    masks_d = nc.declare_dram_parameter("masks", [P, 4 * 2 * QT], at_dt,
                                        isOutput=False)
    yt_d = nc.declare_dram_parameter("yt", [D, T], at_dt, isOutput=True)

    xt_t = xt_d.rearrange("(a p) t -> a p t", p=P)
    wqkt_t = wqkt_d.rearrange("(a p) m -> a p m", p=P)
    wvt_t = wvt_d.rearrange("(a p) m -> a p m", p=P)
    wot_t = wot_d.rearrange("(a p) m -> a p m", p=P)
    yt_t = yt_d.rearrange("(a p) t -> a p t", p=P)

    with nc.allow_low_precision(reason="bf16/f32r compute is intentional"), \
            tile.TileContext(nc) as tc:
        with (
            tc.tile_pool(name="persist", bufs=1) as persist,
            tc.tile_pool(name="xw", bufs=1) as xw,
            tc.tile_pool(name="aps_pool", bufs=2, space="PSUM") as aps_pool,
            tc.tile_pool(name="sc_ps", bufs=2, space="PSUM") as sc_ps,
            tc.tile_pool(name="pv_ps", bufs=1, space="PSUM") as pv_ps,
            tc.tile_pool(name="u_sb", bufs=8) as u_pool,
            tc.tile_pool(name="norm", bufs=1) as norm_pool,
            tc.tile_pool(name="y_sb", bufs=2) as y_pool,
        ):
            # ---- persistent tiles ----
            xt = [xw.tile([P, T], at_dt, name=f"xt{i}", tag=f"xt{i}")
                  for i in range(K_D)]
            wqkt = [xw.tile([P, QK_ROWS], at_dt, name=f"wqkt{i}", tag=f"wqkt{i}")
                    for i in range(K_D)]
            wvt = [xw.tile([P, V_COLS], at_dt, name=f"wvt{i}", tag=f"wvt{i}")
                   for i in range(K_D)]
            masks = persist.tile([P, 8 * QT], at_dt, name="masks", tag="masks")
            wot = [persist.tile([P, D], at_dt, name=f"wot{i}", tag=f"wot{i}")
                   for i in range(C // P)]
            qkt = [persist.tile([P, T], at_dt, name=f"qkt{m}", tag=f"qkt{m}")
                   for m in range(QK_ROWS // P)]
            # v_pad[m]: [128, 4 heads * 128]; per head cols 0:64 = v, 64:128
            # stay at the memset value 1.0 -> PV matmul reproduces Z on psum
            # partitions 64..127.
            v_pad = [persist.tile([P, H_per_core * 2 * hd], at_dt,
                                  name=f"vp{m}", tag=f"vp{m}")
                     for m in range(T // P)]
            attn = [persist.tile([P, T], at_dt, name=f"attn{p}", tag=f"attn{p}")
                    for p in range(n_pairs)]

            # ones columns for the Z trick (GpSimd is otherwise idle, and this
            # keeps Vector free for the first evictions)
            for m in range(T // P):
                nc.gpsimd.memset(v_pad[m][:], 1.0)

            # ---- input DMAs, ordered so the first matmul can start early.
            # Dispatch cost (~150ns per dma_start, plus ring-availability
            # waits) gates the startup, so spread issue across the three
            # DMA-capable sequencers. A sequencer blocks on its own later
            # dispatches, so Scalar (which must run exp from ~13us) gets ONLY
            # the first group; GpSimd (otherwise idle) takes the late chunks.
            for i in range(K_D):     # first t-chunk of x + qk weights
                nc.scalar.dma_start(out=xt[i][:, 0:QT], in_=xt_t[i][:, 0:QT])
                nc.sync.dma_start(out=wqkt[i][:], in_=wqkt_t[i])
            for i in range(K_D):
                nc.sync.dma_start(out=wvt[i][:], in_=wvt_t[i])
            nc.sync.dma_start(out=masks[:], in_=masks_d[:])
            for i in range(K_D):
                nc.gpsimd.dma_start(out=xt[i][:, QT:2 * QT],
                                    in_=xt_t[i][:, QT:2 * QT])
            for i in range(C // P):
                nc.sync.dma_start(out=wot[i][:], in_=wot_t[i])
            for n in range(2, T // QT):
                for i in range(K_D):
                    eng = nc.gpsimd if i % 2 == 0 else nc.sync
                    eng.dma_start(out=xt[i][:, n * QT:(n + 1) * QT],
                                  in_=xt_t[i][:, n * QT:(n + 1) * QT])

            def emit_a(n):
                """qkT + v projection for t-chunk n (QT cols)."""
                for m in range(QK_ROWS // P):
                    ps = aps_pool.tile([P, QT], f32, name="qkps", tag="aps")
                    for k in range(K_D):
                        nc.tensor.matmul(
                            ps[:],
                            lhsT=wqkt[k][:, m * P:(m + 1) * P],
                            rhs=xt[k][:, n * QT:(n + 1) * QT],
                            start=(k == 0), stop=(k == K_D - 1),
                        )
                    nc.vector.tensor_copy(qkt[m][:, n * QT:(n + 1) * QT], ps[:])
                for m in range(4 * n, 4 * n + 4):
                    ps = aps_pool.tile([P, V_COLS], f32, name="vps", tag="aps")
                    for k in range(K_D):
                        nc.tensor.matmul(
                            ps[:],
                            lhsT=xt[k][:, m * P:(m + 1) * P],
                            rhs=wvt[k][:],
                            start=(k == 0), stop=(k == K_D - 1),
                        )
                    # one strided eviction for all 4 heads
                    dst = v_pad[m].rearrange("p (h c) -> p h c", c=2 * hd)[:, :, 0:hd]
                    src = ps.rearrange("p (h c) -> p h c", c=hd)
                    nc.vector.tensor_copy(dst, src)

            def emit_b(g):
                """attention for q-tile g, software-pipelined over s-chunks."""
                q0 = g * QT
                n_sc = (q0 + QT) // SC
                for p in range(n_pairs):
                    qT = qkt[p]        # heads (2p, 2p+1) on partitions 0:64, 64:128
                    kT = qkt[n_pairs + p]
                    # one contiguous 2-bank psum tile: head e in columns
                    # e*QT..(e+1)*QT (each exactly one bank)
                    pv = pv_ps.tile([P, 2 * QT], f32, name="pv", tag="pv")

                    def emit_sc(j):
                        """score matmuls + exp + mask for s-chunk j; returns
                        (u tile, w0)."""
                        s0 = j * SC
                        r = (s0 - q0) // SC
                        w0 = max(r, 0) * SC  # leading fully-masked columns
                        sc = sc_ps.tile([P, 2 * QT], f32, name="sc", tag="sc")
                        for e in range(2):
                            lo, hi = e * 64, e * 64 + 64
                            nc.tensor.matmul(
                                sc[:, e * QT + w0:(e + 1) * QT],
                                lhsT=kT[lo:hi, s0:s0 + SC],
                                rhs=qT[lo:hi, q0 + w0:q0 + QT],
                                start=True, stop=True,
                            )
                        u = u_pool.tile([P, 2 * QT], at_dt, name="u", tag="u")
                        if w0 == 0:
                            nc.scalar.activation(
                                u[:], sc[:], mybir.ActivationFunctionType.Exp,
                                scale=1.0 / math.sqrt(hd),
                            )
                        else:
                            # one strided op covering both heads
                            uv = u.rearrange("p (e q) -> p e q", q=QT)[:, :, w0:QT]
                            scv = sc.rearrange("p (e q) -> p e q", q=QT)[:, :, w0:QT]
                            nc.scalar.activation(
                                uv, scv, mybir.ActivationFunctionType.Exp,
                                scale=1.0 / math.sqrt(hd),
                            )
                        if r >= 0:  # mask the 128-wide triangle block per head
                            uv = u.rearrange("p (e q) -> p e q",
                                             q=QT)[:, :, w0:w0 + SC]
                            mb = masks[:, r * 2 * QT:(r + 1) * 2 * QT].rearrange(
                                "p (e q) -> p e q", q=QT)[:, :, w0:w0 + SC]
                            nc.vector.tensor_mul(uv, uv, mb)
                        return u, w0

                    def emit_pv(j, u, w0):
                        for e in range(2):
                            h = 2 * p + e
                            nc.tensor.matmul(
                                pv[:, e * QT + w0:(e + 1) * QT],
                                lhsT=v_pad[j][:, h * 2 * hd:(h + 1) * 2 * hd],
                                rhs=u[:, e * QT + w0:(e + 1) * QT],
                                start=(j == 0), stop=(j == n_sc - 1),
                            )

                    pend = emit_sc(0)
                    for j in range(n_sc):
                        cur = pend
                        if j + 1 < n_sc:
                            pend = emit_sc(j + 1)
                        emit_pv(j, *cur)

                    # Z sits replicated on psum rows 64..127 of both banks.
                    # 1/Z = exp(-ln Z) on the Scalar engine (ln+exp share the
                    # natural_log_exp_and_others table set; Scalar reads PSUM
                    # directly), so Vector only runs the two fused
                    # normalize-evict multiplies.
                    lz = norm_pool.tile([hd, 2 * QT], f32, name="lz",
                                        tag="lz", bufs=3)
                    rr = norm_pool.tile([hd, 2 * QT], f32, name="rr",
                                        tag="rr", bufs=3)
                    for e in range(2):
                        sl = slice(e * QT, (e + 1) * QT)
                        nc.scalar.activation(
                            lz[0:hd, sl], pv[hd:2 * hd, sl],
                            mybir.ActivationFunctionType.Ln)
                        nc.scalar.activation(
                            rr[0:hd, sl], lz[0:hd, sl],
                            mybir.ActivationFunctionType.Exp, scale=-1.0)
                        nc.vector.tensor_mul(
                            attn[p][e * hd:(e + 1) * hd, q0:q0 + QT],
                            pv[0:hd, sl], rr[0:hd, sl])

            def emit_y(g):
                """out projection for q-tile g."""
                for m in range(D // P):
                    ps = aps_pool.tile([P, QT], f32, name="yps", tag="aps")
                    for k in range(C // P):
                        nc.tensor.matmul(
                            ps[:],
                            lhsT=wot[k][:, m * P:(m + 1) * P],
                            rhs=attn[k][:, g * QT:(g + 1) * QT],
                            start=(k == 0), stop=(k == C // P - 1),
                        )
                    y = y_pool.tile([P, QT], at_dt, name="y", tag="y", bufs=6)
                    nc.vector.tensor_copy(y[:], ps[:])
                    nc.sync.dma_start(
                        out=yt_t[m][:, g * QT:(g + 1) * QT], in_=y[:])

            # ---- pipelined emission ----
            emit_a(0)
            emit_b(0)
            for g in range(1, n_qt):
                emit_a(g)
                emit_y(g - 1)
                emit_b(g)
            emit_y(n_qt - 1)

    nc.finalize()
    return nc


def _make_masks(QT=512, SC=128):
    """Doubled causal masks: [128, 4*2*QT]; block r holds the mask for
    relative offset r twice side by side (head A | head B)."""
    i = np.arange(SC)[:, None]
    j = np.arange(QT)[None, :]
    blocks = []
    for r in range(4):
        m = (i + r * SC <= j).astype(np.float32)
        blocks += [m, m]
    return np.concatenate(blocks, axis=1)


def _cast(arr, dtype_name):
    if dtype_name == "bfloat16":
        import ml_dtypes
        return np.ascontiguousarray(arr.astype(ml_dtypes.bfloat16))
    return np.ascontiguousarray(arr.astype(np.float32))


def kernel(x, Wqkv, Wout, a, b, dt):
    from concourse.bass_utils import run_bass_kernel_spmd

    x = np.asarray(x, dtype=np.float32)
    Wqkv = np.asarray(Wqkv, dtype=np.float32)
    Wout = np.asarray(Wout, dtype=np.float32)
    B, T, D = x.shape
    H, hd = N_HEADS, HEAD_DIM
    hpc = HEADS_PER_CORE
    cores_per_batch = H // hpc
    f0 = _fhn_scale(a, b, dt)

    key = (T, D, hpc, hd)
    if key not in _PROGRAM_CACHE:
        _PROGRAM_CACHE[key] = _build_program(*key)
    nc = _PROGRAM_CACHE[key]

    masks = _cast(_make_masks(), ATTN_DTYPE)
    in_maps = []
    for c in range(N_CORES):
        bi = c // cores_per_batch
        heads = range((c % cores_per_batch) * hpc, (c % cores_per_batch) * hpc + hpc)
        q_rows = np.concatenate([np.arange(h * hd, (h + 1) * hd) for h in heads])
        xt = _cast(x[bi].T, ATTN_DTYPE)                          # (D, T)
        wqk = np.concatenate([Wqkv[q_rows], Wqkv[D + q_rows]], axis=0)
        wqkt = _cast(wqk.T, ATTN_DTYPE)                          # (D, 2*hpc*hd)
        wvt = _cast(Wqkv[2 * D + q_rows].T, ATTN_DTYPE)          # (D, hpc*hd)
        wo = (Wout[:, q_rows].astype(np.float64) * f0).astype(np.float32)
        wot = _cast(wo.T, ATTN_DTYPE)                            # (hpc*hd, D)
        in_maps.append({"xt": xt, "wqkt": wqkt, "wvt": wvt, "wot": wot,
                        "masks": masks})

    trace_dir = os.environ.get("KERNEL_TRACE", "")
    kwargs = {}
    if trace_dir:
        os.makedirs(trace_dir, exist_ok=True)
        kwargs = {"trace": True, "tmpdir": trace_dir}
    res = run_bass_kernel_spmd(nc, in_maps, list(range(N_CORES)), **kwargs)
    LAST_RUN["exec_time_ns"] = res.exec_time_ns
    LAST_RUN["profile_json"] = res.profile_json

    out = np.zeros((B, T, D), dtype=np.float32)
    for bi in range(B):
        acc = np.zeros((D, T), dtype=np.float32)
        for c in range(bi * cores_per_batch, (bi + 1) * cores_per_batch):
            acc += res.results[c]["yt"].astype(np.float32)
        out[bi] = acc.T
    return out


# revision 14
# speedup vs baseline: 1.2378x; 1.2378x over previous
# Trainium2 Bass kernel for nn_FHNTritonAttention: causal attention with an
# FHN (FitzHugh-Nagumo) gate on the attention probabilities.
#
# Math note that shapes the whole kernel: attn_energy = softmax(scores).sum(-1)
# is ~1.0 for every row (softmax rows sum to 1), so stimulus_normed == 1,
# threshold_gate == sigmoid(5), and the FHN recurrence collapses to one
# constant per run. The gate multiplies each probability row by a constant c
# and the subsequent renormalization divides it back out:
#   p'' = p*c / (c*S + 1e-8) = p / (S + 1e-8/c),  S = row sum ~= 1.
# So the entire FHN block reduces to scaling the output by
# f0 = 1/(1 + 1e-8/c0), computed on host from (a, b, dt) and folded into Wout.
#
# Device kernel (SPMD over 8 cores; core = (batch, 4-head group)), emitted as
# one software-pipelined stream so the in-order PE queue never drains:
#   A(n): qkT = Wqk_slice @ x.T (feature on partitions) and v = x @ Wv_slice.T
#         for t-chunk n (512 cols), contraction over D in 8 psum passes.
#         v lands in v_pad tiles [128, h*128] whose upper 64 columns per head
#         are constant 1.0 (memset at start): the PV matmul then produces the
#         softmax denominator Z replicated on psum partitions 64..127.
#   B(g): attention for q-tile g. Per head pair, s-chunks run through a
#         depth-2 software pipeline (score matmul for chunk j+1 issues before
#         the PV matmul of chunk j) so the exp latency on the Scalar engine is
#         hidden. Score matmuls skip the fully-masked leading w0 columns of
#         diagonal chunks. Normalization: one DVE reciprocal of psum rows
#         64..127 ([64, 512]) + one fused multiply that evicts+normalizes the
#         psum into the attn tile — no partition broadcast, no zrow copies.
#   Y(g): out-projection of q-tile g -> DMA out. Emission order
#         A0 B0 A1 Y0 B1 A2 Y1 B2 A3 Y2 B3 Y3 keeps every engine streaming.
#   Host sums the 4 partial products per batch and transposes back.
#
# Matmuls run in bf16 (inputs pre-cast on host; fp32 PSUM accumulation).

import math
import os

import numpy as np

N_HEADS = 16
HEAD_DIM = 64
THRESHOLD = 0.5
TAU = 12.5
N_FHN_STEPS = 2

N_CORES = 8
HEADS_PER_CORE = 4  # cores 0-3 -> batch 0, cores 4-7 -> batch 1

ATTN_DTYPE = os.environ.get("KERNEL_ATTN_DTYPE", "bfloat16")

LAST_RUN = {}  # filled with exec_time_ns etc. when KERNEL_TRACE is set

_PROGRAM_CACHE = {}


def _fhn_scale(a, b, dt):
    """Host-side replica of the reference's gate math at attn_energy == 1."""
    a = float(a)
    b = float(b)
    dt = float(dt)
    sig5 = 1.0 / (1.0 + math.exp(-(1.0 - THRESHOLD) * 10.0))
    i0 = 1.0 * (0.1 + 0.9 * sig5)
    v = 0.0
    w = 0.0
    for _ in range(N_FHN_STEPS):
        v = v + dt * (v - v**3 / 3.0 - w + i0)
        w = (w + (dt / TAU) * (v + a)) / (1.0 + (dt / TAU) * b)
    gate = 1.0 / (1.0 + math.exp(-v))
    c0 = 0.5 + 0.5 * gate
    return c0 / (c0 + 1e-8)


def _pin_combined_exp_ln_table():
    """Force the act-table pass to satisfy both Exp and Ln from the single
    natural_log_exp_and_others set (one ACT_TABLE_LOAD instead of thrashing
    ~2.7us reloads at every head-pair boundary). Indices into the table list
    are ABI (walrus remaps by position), so other sets are emptied in place
    rather than removed."""
    from concourse import bacc, hw_specs

    if getattr(bacc, "_combined_exp_ln_pinned", False):
        return
    orig = hw_specs.get_activation_tables

    def patched(module_arch):
        tables = dict(orig(module_arch))
        import concourse.mybir as mybir
        exp = mybir.ActivationFunctionType.Exp
        ln = mybir.ActivationFunctionType.Ln
        for name in tables:
            if name == "natural_log_exp_and_others":
                continue
            if exp in tables[name] or ln in tables[name]:
                tables[name] = set()
        return tables

    bacc.get_activation_tables = patched
    bacc._combined_exp_ln_pinned = True


def _build_program(T, D, H_per_core, hd):
    import concourse.mybir as mybir
    import concourse.tile as tile
    from concourse import bacc

    _pin_combined_exp_ln_table()

    f32 = mybir.dt.float32
    at_dt = getattr(mybir.dt, ATTN_DTYPE)
    P = 128
    QT = 512   # q tile width (free dim of score/PV matmuls)
    SC = 128   # s chunk depth (contraction of PV, partitions of scoresT)
    K_D = D // P
    QK_ROWS = 2 * H_per_core * hd
    V_COLS = H_per_core * hd
    C = H_per_core * hd
    n_qt = T // QT
    n_pairs = H_per_core // 2

    nc = bacc.Bacc("TRN2", target_bir_lowering=False, debug=False,
                   num_devices=N_CORES)

    xt_d = nc.declare_dram_parameter("xt", [D, T], at_dt, isOutput=False)
    wqkt_d = nc.declare_dram_parameter("wqkt", [D, QK_ROWS], at_dt, isOutput=False)
    wvt_d = nc.declare_dram_parameter("wvt", [D, V_COLS], at_dt, isOutput=False)
    wot_d = nc.declare_dram_parameter("wot", [C, D], at_dt, isOutput=False)
    masks_d = nc.declare_dram_parameter("masks", [P, 4 * 2 * QT], at_dt,
                                        isOutput=False)
    yt_d = nc.declare_dram_parameter("yt", [D, T], at_dt, isOutput=True)

    xt_t = xt_d.rearrange("(a p) t -> a p t", p=P)
    wqkt_t = wqkt_d.rearrange("(a p) m -> a p m", p=P)
    wvt_t = wvt_d.rearrange("(a p) m -> a p m", p=P)
    wot_t = wot_d.rearrange("(a p) m -> a p m", p=P)
    yt_t = yt_d.rearrange("(a p) t -> a p t", p=P)

    with nc.allow_low_precision(reason="bf16/f32r compute is intentional"), \
            tile.TileContext(nc) as tc:
        with (
            tc.tile_pool(name="persist", bufs=1) as persist,
            tc.tile_pool(name="xw", bufs=1) as xw,
            tc.tile_pool(name="aps_pool", bufs=2, space="PSUM") as aps_pool,
            tc.tile_pool(name="sc_ps", bufs=2, space="PSUM") as sc_ps,
            tc.tile_pool(name="pv_ps", bufs=1, space="PSUM") as pv_ps,
            tc.tile_pool(name="u_sb", bufs=8) as u_pool,
            tc.tile_pool(name="norm", bufs=1) as norm_pool,
            tc.tile_pool(name="y_sb", bufs=2) as y_pool,
        ):
            # ---- persistent tiles ----
            xt = [xw.tile([P, T], at_dt, name=f"xt{i}", tag=f"xt{i}")
                  for i in range(K_D)]
            wqkt = [xw.tile([P, QK_ROWS], at_dt, name=f"wqkt{i}", tag=f"wqkt{i}")
                    for i in range(K_D)]
            wvt = [xw.tile([P, V_COLS], at_dt, name=f"wvt{i}", tag=f"wvt{i}")
                   for i in range(K_D)]
            masks = persist.tile([P, 8 * QT], at_dt, name="masks", tag="masks")
            wot = [persist.tile([P, D], at_dt, name=f"wot{i}", tag=f"wot{i}")
                   for i in range(C // P)]
            qkt = [persist.tile([P, T], at_dt, name=f"qkt{m}", tag=f"qkt{m}")
                   for m in range(QK_ROWS // P)]
            # v_pad[m]: [128, 4 heads * 128]; per head cols 0:64 = v, 64:128
            # stay at the memset value 1.0 -> PV matmul reproduces Z on psum
            # partitions 64..127.
            v_pad = [persist.tile([P, H_per_core * 2 * hd], at_dt,
                                  name=f"vp{m}", tag=f"vp{m}")
                     for m in range(T // P)]
            attn = [persist.tile([P, T], at_dt, name=f"attn{p}", tag=f"attn{p}")
                    for p in range(n_pairs)]

            # ones columns for the Z trick (GpSimd is otherwise idle, and this
            # keeps Vector free for the first evictions)
            for m in range(T // P):
                nc.gpsimd.memset(v_pad[m][:], 1.0)

            # ---- input DMAs, ordered so the first matmul can start early.
            # Dispatch cost (~150ns per dma_start, plus ring-availability
            # waits) gates the startup, so spread issue across the three
            # DMA-capable sequencers. A sequencer blocks on its own later
            # dispatches, so Scalar (which must run exp from ~13us) gets ONLY
            # the first group; GpSimd (otherwise idle) takes the late chunks.
            for i in range(K_D):     # first t-chunk of x + qk weights
                nc.scalar.dma_start(out=xt[i][:, 0:QT], in_=xt_t[i][:, 0:QT])
                nc.sync.dma_start(out=wqkt[i][:], in_=wqkt_t[i])
            for i in range(K_D):
                nc.sync.dma_start(out=wvt[i][:], in_=wvt_t[i])
            nc.sync.dma_start(out=masks[:], in_=masks_d[:])
            for i in range(K_D):
                nc.gpsimd.dma_start(out=xt[i][:, QT:2 * QT],
                                    in_=xt_t[i][:, QT:2 * QT])
            for i in range(C // P):
                nc.sync.dma_start(out=wot[i][:], in_=wot_t[i])
            for n in range(2, T // QT):
                for i in range(K_D):
                    eng = nc.gpsimd if i % 2 == 0 else nc.sync
                    eng.dma_start(out=xt[i][:, n * QT:(n + 1) * QT],
                                  in_=xt_t[i][:, n * QT:(n + 1) * QT])

            def emit_a(n):
                """qkT + v projection for t-chunk n (QT cols)."""
                for m in range(QK_ROWS // P):
                    ps = aps_pool.tile([P, QT], f32, name="qkps", tag="aps")
                    for k in range(K_D):
                        nc.tensor.matmul(
                            ps[:],
                            lhsT=wqkt[k][:, m * P:(m + 1) * P],
                            rhs=xt[k][:, n * QT:(n + 1) * QT],
                            start=(k == 0), stop=(k == K_D - 1),
                        )
                    nc.vector.tensor_copy(qkt[m][:, n * QT:(n + 1) * QT], ps[:])
                for m in range(4 * n, 4 * n + 4):
                    ps = aps_pool.tile([P, V_COLS], f32, name="vps", tag="aps")
                    for k in range(K_D):
                        nc.tensor.matmul(
                            ps[:],
                            lhsT=xt[k][:, m * P:(m + 1) * P],
                            rhs=wvt[k][:],
                            start=(k == 0), stop=(k == K_D - 1),
                        )
                    # one strided eviction for all 4 heads
                    dst = v_pad[m].rearrange("p (h c) -> p h c", c=2 * hd)[:, :, 0:hd]
                    src = ps.rearrange("p (h c) -> p h c", c=hd)
                    nc.vector.tensor_copy(dst, src)

            def emit_b(g):
                """attention for q-tile g, software-pipelined over s-chunks."""
                q0 = g * QT
                n_sc = (q0 + QT) // SC
                for p in range(n_pairs):
                    qT = qkt[p]        # heads (2p, 2p+1) on partitions 0:64, 64:128
                    kT = qkt[n_pairs + p]
                    # one contiguous 2-bank psum tile: head e in columns
                    # e*QT..(e+1)*QT (each exactly one bank)
                    pv = pv_ps.tile([P, 2 * QT], f32, name="pv", tag="pv")

                    def emit_sc(j):
                        """score matmuls + exp + mask for s-chunk j; returns
                        (u tile, w0)."""
                        s0 = j * SC
                        r = (s0 - q0) // SC
                        w0 = max(r, 0) * SC  # leading fully-masked columns
                        sc = sc_ps.tile([P, 2 * QT], f32, name="sc", tag="sc")
                        for e in range(2):
                            lo, hi = e * 64, e * 64 + 64
                            nc.tensor.matmul(
                                sc[:, e * QT + w0:(e + 1) * QT],
                                lhsT=kT[lo:hi, s0:s0 + SC],
                                rhs=qT[lo:hi, q0 + w0:q0 + QT],
                                start=True, stop=True,
                            )
                        u = u_pool.tile([P, 2 * QT], at_dt, name="u", tag="u")
                        if w0 == 0:
                            nc.scalar.activation(
                                u[:], sc[:], mybir.ActivationFunctionType.Exp,
                                scale=1.0 / math.sqrt(hd),
                            )
                        else:
                            # one strided op covering both heads
                            uv = u.rearrange("p (e q) -> p e q", q=QT)[:, :, w0:QT]
                            scv = sc.rearrange("p (e q) -> p e q", q=QT)[:, :, w0:QT]
                            nc.scalar.activation(
                                uv, scv, mybir.ActivationFunctionType.Exp,
                                scale=1.0 / math.sqrt(hd),
                            )
                        if r >= 0:  # mask the 128-wide triangle block per head
                            uv = u.rearrange("p (e q) -> p e q",
                                             q=QT)[:, :, w0:w0 + SC]
                            mb = masks[:, r * 2 * QT:(r + 1) * 2 * QT].rearrange(
                                "p (e q) -> p e q", q=QT)[:, :, w0:w0 + SC]
                            nc.vector.tensor_mul(uv, uv, mb)
                        return u, w0

                    def emit_pv(j, u, w0):
                        for e in range(2):
                            h = 2 * p + e
                            nc.tensor.matmul(
                                pv[:, e * QT + w0:(e + 1) * QT],
                                lhsT=v_pad[j][:, h * 2 * hd:(h + 1) * 2 * hd],
                                rhs=u[:, e * QT + w0:(e + 1) * QT],
                                start=(j == 0), stop=(j == n_sc - 1),
                            )

                    pend = emit_sc(0)
                    for j in range(n_sc):
                        cur = pend
                        if j + 1 < n_sc:
                            pend = emit_sc(j + 1)
                        emit_pv(j, *cur)

                    # Z sits replicated on psum rows 64..127 of both banks.
                    # 1/Z = exp(-ln Z) on the Scalar engine (ln+exp share the
                    # natural_log_exp_and_others table set; Scalar reads PSUM
                    # directly), so Vector only runs the two fused
                    # normalize-evict multiplies.
                    lz = norm_pool.tile([hd, 2 * QT], f32, name="lz",
                                        tag="lz", bufs=3)
                    rr = norm_pool.tile([hd, 2 * QT], f32, name="rr",
                                        tag="rr", bufs=3)
                    for e in range(2):
                        sl = slice(e * QT, (e + 1) * QT)
                        nc.scalar.activation(
                            lz[0:hd, sl], pv[hd:2 * hd, sl],
                            mybir.ActivationFunctionType.Ln)
                        nc.scalar.activation(
                            rr[0:hd, sl], lz[0:hd, sl],
                            mybir.ActivationFunctionType.Exp, scale=-1.0)
                        nc.vector.tensor_mul(
                            attn[p][e * hd:(e + 1) * hd, q0:q0 + QT],
                            pv[0:hd, sl], rr[0:hd, sl])

            def emit_y(g):
                """out projection for q-tile g."""
                for m in range(D // P):
                    ps = aps_pool.tile([P, QT], f32, name="yps", tag="aps")
                    for k in range(C // P):
                        nc.tensor.matmul(
                            ps[:],
                            lhsT=wot[k][:, m * P:(m + 1) * P],
                            rhs=attn[k][:, g * QT:(g + 1) * QT],
                            start=(k == 0), stop=(k == C // P - 1),
                        )
                    y = y_pool.tile([P, QT], at_dt, name="y", tag="y", bufs=6)
                    nc.vector.tensor_copy(y[:], ps[:])
                    nc.sync.dma_start(
                        out=yt_t[m][:, g * QT:(g + 1) * QT], in_=y[:])

            # ---- pipelined emission ----
            emit_a(0)
            emit_b(0)
            for g in range(1, n_qt):
                emit_a(g)
                emit_y(g - 1)
                emit_b(g)
            emit_y(n_qt - 1)

    nc.finalize()
    return nc


def _make_masks(QT=512, SC=128):
    """Doubled causal masks: [128, 4*2*QT]; block r holds the mask for
    relative offset r twice side by side (head A | head B)."""
    i = np.arange(SC)[:, None]
    j = np.arange(QT)[None, :]
    blocks = []
    for r in range(4):
        m = (i + r * SC <= j).astype(np.float32)
        blocks += [m, m]
    return np.concatenate(blocks, axis=1)


def _cast(arr, dtype_name):
    if dtype_name == "bfloat16":
        import ml_dtypes
        return np.ascontiguousarray(arr.astype(ml_dtypes.bfloat16))
    return np.ascontiguousarray(arr.astype(np.float32))


def kernel(x, Wqkv, Wout, a, b, dt):
    from concourse.bass_utils import run_bass_kernel_spmd

    x = np.asarray(x, dtype=np.float32)
    Wqkv = np.asarray(Wqkv, dtype=np.float32)
    Wout = np.asarray(Wout, dtype=np.float32)
    B, T, D = x.shape
    H, hd = N_HEADS, HEAD_DIM
    hpc = HEADS_PER_CORE
    cores_per_batch = H // hpc
    f0 = _fhn_scale(a, b, dt)

    key = (T, D, hpc, hd)
    if key not in _PROGRAM_CACHE:
        _PROGRAM_CACHE[key] = _build_program(*key)
    nc = _PROGRAM_CACHE[key]

    masks = _cast(_make_masks(), ATTN_DTYPE)
    in_maps = []
    for c in range(N_CORES):
        bi = c // cores_per_batch
        heads = range((c % cores_per_batch) * hpc, (c % cores_per_batch) * hpc + hpc)
        q_rows = np.concatenate([np.arange(h * hd, (h + 1) * hd) for h in heads])
        xt = _cast(x[bi].T, ATTN_DTYPE)                          # (D, T)
        wqk = np.concatenate([Wqkv[q_rows], Wqkv[D + q_rows]], axis=0)
        wqkt = _cast(wqk.T, ATTN_DTYPE)                          # (D, 2*hpc*hd)
        wvt = _cast(Wqkv[2 * D + q_rows].T, ATTN_DTYPE)          # (D, hpc*hd)
        wo = (Wout[:, q_rows].astype(np.float64) * f0).astype(np.float32)
        wot = _cast(wo.T, ATTN_DTYPE)                            # (hpc*hd, D)
        in_maps.append({"xt": xt, "wqkt": wqkt, "wvt": wvt, "wot": wot,
                        "masks": masks})

    trace_dir = os.environ.get("KERNEL_TRACE", "")
    kwargs = {}
    if trace_dir:
        os.makedirs(trace_dir, exist_ok=True)
        kwargs = {"trace": True, "tmpdir": trace_dir}
    res = run_bass_kernel_spmd(nc, in_maps, list(range(N_CORES)), **kwargs)
    LAST_RUN["exec_time_ns"] = res.exec_time_ns
    LAST_RUN["profile_json"] = res.profile_json

    out = np.zeros((B, T, D), dtype=np.float32)
    for bi in range(B):
        acc = np.zeros((D, T), dtype=np.float32)
        for c in range(bi * cores_per_batch, (bi + 1) * cores_per_batch):
            acc += res.results[c]["yt"].astype(np.float32)
        out[bi] = acc.T
    return out


# revision 15
# speedup vs baseline: 1.3014x; 1.0513x over previous
# Trainium2 Bass kernel for nn_FHNTritonAttention: causal attention with an
# FHN (FitzHugh-Nagumo) gate on the attention probabilities.
#
# Math note that shapes the whole kernel: attn_energy = softmax(scores).sum(-1)
# is ~1.0 for every row (softmax rows sum to 1), so stimulus_normed == 1,
# threshold_gate == sigmoid(5), and the FHN recurrence collapses to one
# constant per run. The gate multiplies each probability row by a constant c
# and the subsequent renormalization divides it back out:
#   p'' = p*c / (c*S + 1e-8) = p / (S + 1e-8/c),  S = row sum ~= 1.
# So the entire FHN block reduces to scaling the output by
# f0 = 1/(1 + 1e-8/c0), computed on host from (a, b, dt) and folded into Wout.
#
# Device kernel (SPMD over 8 cores; core = (batch, 4-head group)), emitted as
# one software-pipelined stream so the in-order PE queue never drains:
#   A(n): qkT = Wqk_slice @ x.T (feature on partitions) and v = x @ Wv_slice.T
#         for t-chunk n (512 cols), contraction over D in 8 psum passes.
#         v lands in v_pad tiles [128, h*128] whose upper 64 columns per head
#         are constant 1.0 (memset at start): the PV matmul then produces the
#         softmax denominator Z replicated on psum partitions 64..127.
#   B(g): attention for q-tile g. Per head pair, s-chunks run through a
#         depth-2 software pipeline (score matmul for chunk j+1 issues before
#         the PV matmul of chunk j) so the exp latency on the Scalar engine is
#         hidden. Score matmuls skip the fully-masked leading w0 columns of
#         diagonal chunks. Normalization: one DVE reciprocal of psum rows
#         64..127 ([64, 512]) + one fused multiply that evicts+normalizes the
#         psum into the attn tile — no partition broadcast, no zrow copies.
#   Y(g): out-projection of q-tile g -> DMA out. Emission order
#         A0 B0 A1 Y0 B1 A2 Y1 B2 A3 Y2 B3 Y3 keeps every engine streaming.
#   Host sums the 4 partial products per batch and transposes back.
#
# Matmuls run in bf16 (inputs pre-cast on host; fp32 PSUM accumulation).

import math
import os

import numpy as np

N_HEADS = 16
HEAD_DIM = 64
THRESHOLD = 0.5
TAU = 12.5
N_FHN_STEPS = 2

N_CORES = 8
HEADS_PER_CORE = 4  # cores 0-3 -> batch 0, cores 4-7 -> batch 1

ATTN_DTYPE = os.environ.get("KERNEL_ATTN_DTYPE", "bfloat16")

LAST_RUN = {}  # filled with exec_time_ns etc. when KERNEL_TRACE is set

_PROGRAM_CACHE = {}


def _fhn_scale(a, b, dt):
    """Host-side replica of the reference's gate math at attn_energy == 1."""
    a = float(a)
    b = float(b)
    dt = float(dt)
    sig5 = 1.0 / (1.0 + math.exp(-(1.0 - THRESHOLD) * 10.0))
    i0 = 1.0 * (0.1 + 0.9 * sig5)
    v = 0.0
    w = 0.0
    for _ in range(N_FHN_STEPS):
        v = v + dt * (v - v**3 / 3.0 - w + i0)
        w = (w + (dt / TAU) * (v + a)) / (1.0 + (dt / TAU) * b)
    gate = 1.0 / (1.0 + math.exp(-v))
    c0 = 0.5 + 0.5 * gate
    return c0 / (c0 + 1e-8)


def _pin_combined_exp_ln_table():
    """Force the act-table pass to satisfy both Exp and Ln from the single
    natural_log_exp_and_others set (one ACT_TABLE_LOAD instead of thrashing
    ~2.7us reloads at every head-pair boundary). Indices into the table list
    are ABI (walrus remaps by position), so other sets are emptied in place
    rather than removed."""
    from concourse import bacc, hw_specs

    if getattr(bacc, "_combined_exp_ln_pinned", False):
        return
    orig = hw_specs.get_activation_tables

    def patched(module_arch):
        tables = dict(orig(module_arch))
        import concourse.mybir as mybir
        exp = mybir.ActivationFunctionType.Exp
        ln = mybir.ActivationFunctionType.Ln
        for name in tables:
            if name == "natural_log_exp_and_others":
                continue
            if exp in tables[name] or ln in tables[name]:
                tables[name] = set()
        return tables

    bacc.get_activation_tables = patched
    bacc._combined_exp_ln_pinned = True


def _build_program(T, D, H_per_core, hd):
    import concourse.mybir as mybir
    import concourse.tile as tile
    from concourse import bacc

    _pin_combined_exp_ln_table()

    f32 = mybir.dt.float32
    at_dt = getattr(mybir.dt, ATTN_DTYPE)
    P = 128
    QT = 512   # q tile width (free dim of score/PV matmuls)
    SC = 128   # s chunk depth (contraction of PV, partitions of scoresT)
    K_D = D // P
    QK_ROWS = 2 * H_per_core * hd
    V_COLS = H_per_core * hd
    C = H_per_core * hd
    n_qt = T // QT
    n_pairs = H_per_core // 2

    nc = bacc.Bacc("TRN2", target_bir_lowering=False, debug=False,
                   num_devices=N_CORES)

    xt_d = nc.declare_dram_parameter("xt", [D, T], at_dt, isOutput=False)
    wqkt_d = nc.declare_dram_parameter("wqkt", [D, QK_ROWS], at_dt, isOutput=False)
    wvt_d = nc.declare_dram_parameter("wvt", [D, V_COLS], at_dt, isOutput=False)
    wot_d = nc.declare_dram_parameter("wot", [C, D], at_dt, isOutput=False)
    masks_d = nc.declare_dram_parameter("masks", [P, 4 * 2 * QT], at_dt,
                                        isOutput=False)
    yt_d = nc.declare_dram_parameter("yt", [D, T], at_dt, isOutput=True)

    xt_t = xt_d.rearrange("(a p) t -> a p t", p=P)
    wqkt_t = wqkt_d.rearrange("(a p) m -> a p m", p=P)
    wvt_t = wvt_d.rearrange("(a p) m -> a p m", p=P)
    wot_t = wot_d.rearrange("(a p) m -> a p m", p=P)
    yt_t = yt_d.rearrange("(a p) t -> a p t", p=P)

    with nc.allow_low_precision(reason="bf16/f32r compute is intentional"), \
            tile.TileContext(nc) as tc:
        with (
            tc.tile_pool(name="persist", bufs=1) as persist,
            tc.tile_pool(name="xw", bufs=1) as xw,
            tc.tile_pool(name="aps_pool", bufs=2, space="PSUM") as aps_pool,
            tc.tile_pool(name="sc_ps", bufs=2, space="PSUM") as sc_ps,
            tc.tile_pool(name="pv_ps", bufs=1, space="PSUM") as pv_ps,
            tc.tile_pool(name="u_sb", bufs=8) as u_pool,
            tc.tile_pool(name="norm", bufs=1) as norm_pool,
            tc.tile_pool(name="y_sb", bufs=2) as y_pool,
        ):
            # ---- persistent tiles ----
            xt = [xw.tile([P, T], at_dt, name=f"xt{i}", tag=f"xt{i}")
                  for i in range(K_D)]
            wqkt = [xw.tile([P, QK_ROWS], at_dt, name=f"wqkt{i}", tag=f"wqkt{i}")
                    for i in range(K_D)]
            wvt = [xw.tile([P, V_COLS], at_dt, name=f"wvt{i}", tag=f"wvt{i}")
                   for i in range(K_D)]
            masks = persist.tile([P, 8 * QT], at_dt, name="masks", tag="masks")
            wot = [persist.tile([P, D], at_dt, name=f"wot{i}", tag=f"wot{i}")
                   for i in range(C // P)]
            qkt = [persist.tile([P, T], at_dt, name=f"qkt{m}", tag=f"qkt{m}")
                   for m in range(QK_ROWS // P)]
            # v_pad[m]: [128, 4 heads * 128]; per head cols 0:64 = v, 64:128
            # stay at the memset value 1.0 -> PV matmul reproduces Z on psum
            # partitions 64..127.
            v_pad = [persist.tile([P, H_per_core * 2 * hd], at_dt,
                                  name=f"vp{m}", tag=f"vp{m}")
                     for m in range(T // P)]
            attn = [persist.tile([P, T], at_dt, name=f"attn{p}", tag=f"attn{p}")
                    for p in range(n_pairs)]

            # ones columns for the Z trick (GpSimd is otherwise idle, and this
            # keeps Vector free for the first evictions)
            for m in range(T // P):
                nc.gpsimd.memset(v_pad[m][:], 1.0)

            # ---- input DMAs, ordered so the first matmul can start early.
            # Dispatch cost (~150ns per dma_start, plus ring-availability
            # waits) gates the startup, so spread issue across the three
            # DMA-capable sequencers. A sequencer blocks on its own later
            # dispatches, so Scalar (which must run exp from ~13us) gets ONLY
            # the first group; GpSimd (otherwise idle) takes the late chunks.
            for i in range(K_D):     # first t-chunk of x + qk weights
                nc.scalar.dma_start(out=xt[i][:, 0:QT], in_=xt_t[i][:, 0:QT])
                nc.sync.dma_start(out=wqkt[i][:], in_=wqkt_t[i])
            for i in range(K_D):
                nc.sync.dma_start(out=wvt[i][:], in_=wvt_t[i])
            nc.sync.dma_start(out=masks[:], in_=masks_d[:])
            for i in range(K_D):
                nc.gpsimd.dma_start(out=xt[i][:, QT:2 * QT],
                                    in_=xt_t[i][:, QT:2 * QT])
            for i in range(C // P):
                nc.sync.dma_start(out=wot[i][:], in_=wot_t[i])
            for n in range(2, T // QT):
                for i in range(K_D):
                    eng = nc.gpsimd if i % 2 == 0 else nc.sync
                    eng.dma_start(out=xt[i][:, n * QT:(n + 1) * QT],
                                  in_=xt_t[i][:, n * QT:(n + 1) * QT])

            def emit_a(n):
                """qkT + v projection for t-chunk n (QT cols)."""
                for m in range(QK_ROWS // P):
                    ps = aps_pool.tile([P, QT], f32, name="qkps", tag="aps")
                    for k in range(K_D):
                        nc.tensor.matmul(
                            ps[:],
                            lhsT=wqkt[k][:, m * P:(m + 1) * P],
                            rhs=xt[k][:, n * QT:(n + 1) * QT],
                            start=(k == 0), stop=(k == K_D - 1),
                        )
                    nc.vector.tensor_copy(qkt[m][:, n * QT:(n + 1) * QT], ps[:])
                for m in range(4 * n, 4 * n + 4):
                    ps = aps_pool.tile([P, V_COLS], f32, name="vps", tag="aps")
                    for k in range(K_D):
                        nc.tensor.matmul(
                            ps[:],
                            lhsT=xt[k][:, m * P:(m + 1) * P],
                            rhs=wvt[k][:],
                            start=(k == 0), stop=(k == K_D - 1),
                        )
                    # one strided eviction for all 4 heads
                    dst = v_pad[m].rearrange("p (h c) -> p h c", c=2 * hd)[:, :, 0:hd]
                    src = ps.rearrange("p (h c) -> p h c", c=hd)
                    nc.vector.tensor_copy(dst, src)

            def emit_b(g):
                """attention for q-tile g, software-pipelined over s-chunks."""
                q0 = g * QT
                n_sc = (q0 + QT) // SC
                for p in range(n_pairs):
                    qT = qkt[p]        # heads (2p, 2p+1) on partitions 0:64, 64:128
                    kT = qkt[n_pairs + p]
                    # one contiguous 2-bank psum tile: head e in columns
                    # e*QT..(e+1)*QT (each exactly one bank)
                    pv = pv_ps.tile([P, 2 * QT], f32, name="pv", tag="pv")

                    def emit_sc(j):
                        """score matmuls + exp + mask for s-chunk j; returns
                        (u tile, w0)."""
                        s0 = j * SC
                        r = (s0 - q0) // SC
                        w0 = max(r, 0) * SC  # leading fully-masked columns
                        sc = sc_ps.tile([P, 2 * QT], f32, name="sc", tag="sc")
                        for e in range(2):
                            lo, hi = e * 64, e * 64 + 64
                            nc.tensor.matmul(
                                sc[:, e * QT + w0:(e + 1) * QT],
                                lhsT=kT[lo:hi, s0:s0 + SC],
                                rhs=qT[lo:hi, q0 + w0:q0 + QT],
                                start=True, stop=True,
                            )
                        u = u_pool.tile([P, 2 * QT], at_dt, name="u", tag="u")
                        if w0 == 0:
                            nc.scalar.activation(
                                u[:], sc[:], mybir.ActivationFunctionType.Exp,
                                scale=1.0 / math.sqrt(hd),
                            )
                        else:
                            # one strided op covering both heads
                            uv = u.rearrange("p (e q) -> p e q", q=QT)[:, :, w0:QT]
                            scv = sc.rearrange("p (e q) -> p e q", q=QT)[:, :, w0:QT]
                            nc.scalar.activation(
                                uv, scv, mybir.ActivationFunctionType.Exp,
                                scale=1.0 / math.sqrt(hd),
                            )
                        if r >= 0:  # mask the 128-wide triangle block per head
                            uv = u.rearrange("p (e q) -> p e q",
                                             q=QT)[:, :, w0:w0 + SC]
                            mb = masks[:, r * 2 * QT:(r + 1) * 2 * QT].rearrange(
                                "p (e q) -> p e q", q=QT)[:, :, w0:w0 + SC]
                            nc.vector.tensor_mul(uv, uv, mb)
                        return u, w0

                    def emit_pv(j, u, w0):
                        for e in range(2):
                            h = 2 * p + e
                            nc.tensor.matmul(
                                pv[:, e * QT + w0:(e + 1) * QT],
                                lhsT=v_pad[j][:, h * 2 * hd:(h + 1) * 2 * hd],
                                rhs=u[:, e * QT + w0:(e + 1) * QT],
                                start=(j == 0), stop=(j == n_sc - 1),
                            )

                    pend = emit_sc(0)
                    for j in range(n_sc):
                        cur = pend
                        if j + 1 < n_sc:
                            pend = emit_sc(j + 1)
                        emit_pv(j, *cur)

                    # Z sits replicated on psum rows 64..127 of both banks:
                    # one wide eviction copy + one wide approx reciprocal,
                    # then one fused normalize-evict multiply per head.
                    zz = norm_pool.tile([hd, 2 * QT], f32, name="zz",
                                        tag="zz", bufs=3)
                    nc.vector.tensor_copy(zz[0:hd, :], pv[hd:2 * hd, :])
                    rr = norm_pool.tile([hd, 2 * QT], f32, name="rr",
                                        tag="rr", bufs=3)
                    nc.vector.reciprocal_approx_fast(
                        out=rr[0:hd, :], in_=zz[0:hd, :])
                    for e in range(2):
                        nc.vector.tensor_mul(
                            attn[p][e * hd:(e + 1) * hd, q0:q0 + QT],
                            pv[0:hd, e * QT:(e + 1) * QT],
                            rr[0:hd, e * QT:(e + 1) * QT])

            def emit_y(g):
                """out projection for q-tile g."""
                for m in range(D // P):
                    ps = aps_pool.tile([P, QT], f32, name="yps", tag="aps")
                    for k in range(C // P):
                        nc.tensor.matmul(
                            ps[:],
                            lhsT=wot[k][:, m * P:(m + 1) * P],
                            rhs=attn[k][:, g * QT:(g + 1) * QT],
                            start=(k == 0), stop=(k == C // P - 1),
                        )
                    y = y_pool.tile([P, QT], at_dt, name="y", tag="y", bufs=6)
                    nc.vector.tensor_copy(y[:], ps[:])
                    nc.sync.dma_start(
                        out=yt_t[m][:, g * QT:(g + 1) * QT], in_=y[:])

            # ---- pipelined emission ----
            emit_a(0)
            emit_b(0)
            for g in range(1, n_qt):
                emit_a(g)
                emit_y(g - 1)
                emit_b(g)
            emit_y(n_qt - 1)

    nc.finalize()
    return nc


def _make_masks(QT=512, SC=128):
    """Doubled causal masks: [128, 4*2*QT]; block r holds the mask for
    relative offset r twice side by side (head A | head B)."""
    i = np.arange(SC)[:, None]
    j = np.arange(QT)[None, :]
    blocks = []
    for r in range(4):
        m = (i + r * SC <= j).astype(np.float32)
        blocks += [m, m]
    return np.concatenate(blocks, axis=1)


def _cast(arr, dtype_name):
    if dtype_name == "bfloat16":
        import ml_dtypes
        return np.ascontiguousarray(arr.astype(ml_dtypes.bfloat16))
    return np.ascontiguousarray(arr.astype(np.float32))


def kernel(x, Wqkv, Wout, a, b, dt):
    from concourse.bass_utils import run_bass_kernel_spmd

    x = np.asarray(x, dtype=np.float32)
    Wqkv = np.asarray(Wqkv, dtype=np.float32)
    Wout = np.asarray(Wout, dtype=np.float32)
    B, T, D = x.shape
    H, hd = N_HEADS, HEAD_DIM
    hpc = HEADS_PER_CORE
    cores_per_batch = H // hpc
    f0 = _fhn_scale(a, b, dt)

    key = (T, D, hpc, hd)
    if key not in _PROGRAM_CACHE:
        _PROGRAM_CACHE[key] = _build_program(*key)
    nc = _PROGRAM_CACHE[key]

    masks = _cast(_make_masks(), ATTN_DTYPE)
    in_maps = []
    for c in range(N_CORES):
        bi = c // cores_per_batch
        heads = range((c % cores_per_batch) * hpc, (c % cores_per_batch) * hpc + hpc)
        q_rows = np.concatenate([np.arange(h * hd, (h + 1) * hd) for h in heads])
        xt = _cast(x[bi].T, ATTN_DTYPE)                          # (D, T)
        wqk = np.concatenate([Wqkv[q_rows], Wqkv[D + q_rows]], axis=0)
        wqkt = _cast(wqk.T, ATTN_DTYPE)                          # (D, 2*hpc*hd)
        wvt = _cast(Wqkv[2 * D + q_rows].T, ATTN_DTYPE)          # (D, hpc*hd)
        wo = (Wout[:, q_rows].astype(np.float64) * f0).astype(np.float32)
        wot = _cast(wo.T, ATTN_DTYPE)                            # (hpc*hd, D)
        in_maps.append({"xt": xt, "wqkt": wqkt, "wvt": wvt, "wot": wot,
                        "masks": masks})

    trace_dir = os.environ.get("KERNEL_TRACE", "")
    kwargs = {}
    if trace_dir:
        os.makedirs(trace_dir, exist_ok=True)
        kwargs = {"trace": True, "tmpdir": trace_dir}
    res = run_bass_kernel_spmd(nc, in_maps, list(range(N_CORES)), **kwargs)
    LAST_RUN["exec_time_ns"] = res.exec_time_ns
    LAST_RUN["profile_json"] = res.profile_json

    out = np.zeros((B, T, D), dtype=np.float32)
    for bi in range(B):
        acc = np.zeros((D, T), dtype=np.float32)
        for c in range(bi * cores_per_batch, (bi + 1) * cores_per_batch):
            acc += res.results[c]["yt"].astype(np.float32)
        out[bi] = acc.T
    return out
